# revision 3
# baseline (speedup 1.0000x reference)
"""Trainium2 Bass kernel for nn_DigitConvolutionalModel (3x3 conv + 3-layer MLP).

Math: out = relu(relu(conv3x3(x) @ W1 + b1) @ W2 + b2) @ W3 + b3.

The 3x3 valid conv is linear, so on host we fold it into the first FC:
  h1 = relu(x @ W1eff + b1)  with  W1eff = A @ W1 : [784, 256].
K = 784 is used EXACTLY (6 full 128-row k-tiles + one 16-row tail tile).
b1 rides the tail matmul as a 17th row (ones row in x-tail, b1 row in
W-tail), so L1 needs no separate bias add.  b2/b3 ride as fp16 columns of
the wa tensor.

Sharding: pure data parallel over the batch across 8 cores (2048 rows each).
Feature-major 3-layer MLP (activations transposed; zero on-device transposes):
  h1T = relu(W1eff.T @ xT [+b1 via ones-row])   [256, 2048]
  h2T = relu(W2.T   @ h1T + b2)                 [256, 2048]
  oT  =      W3.T   @ h2T + b3                  [10, 2048]
Matmuls in fp16 (full-rate PE) with fp32 PSUM accumulation.

DMA/fill discipline (from trace analysis of the previous kernel):
 - Each dma_start costs its issuing sequencer ~0.65us of serial descriptor
   generation, and a queue's first bytes land ~0.8us after descgen ends.
   The HWDGE engines (sync/scalar) only come alive ~1.4us into the span,
   but GpSimd is alive at ~0 -- so the first chunk-0 pieces ride SWDGE.
 - Chunk-0 is split into SELF-CONTAINED per-k pieces [w1_k | x0_k] so each
   piece's semaphore releases both matmuls of that k immediately.  Pieces
   are spread round-robin over gpsimd/sync/scalar so no queue builds a
   backlog that delays another queue's first transfer.
 - The PE HAM clock gate holds the PE at 1.2GHz until ~3.4us of sustained
   busy-ness; a short warmup burst bridges engine-alive -> first data, and
   early real matmuls (still cold) keep accumulating busy time.
 - Later chunks prefetch in 3-k-tile halves on the two HWDGE rings
   (bufs=3 so no dma_start ever blocks its sequencer on buffer reuse).
 - Output is stored as fp16 (cast up on host; adds ~1e-4 rel err) and the
   last chunk's L3/bias/store is split into two 256-col halves so the
   exposed post-matmul tail is short.
"""

import numpy as np

import concourse.bacc as bacc
import concourse.bass as bass
import concourse.mybir as mybir
import concourse.tile as tile
from concourse.bass_utils import run_bass_kernel_spmd

N_CORES = 8
B = 16384
B_LOC = B // N_CORES  # 2048 batch rows per core
NCH = 512  # batch chunk per matmul (fp32 PSUM bank = 512 floats)
NCHUNKS = B_LOC // NCH
KIN = 784  # folded input features (28*28)
NK = 6  # full 128-row k-tiles; tile 6 is the 16-row tail (+1 ones row)
KTAIL = KIN - NK * 128  # 16
H = 256
NOUT = 10
NWARM = 10  # small PE warm-up matmuls bridging engine-alive -> first data

WA_W3 = 2 * H  # col offset of w3 block in wa
WA_B2 = WA_W3 + 2 * NOUT  # col offset of b2 (2 cols)
WA_B3 = WA_B2 + 2  # col offset of b3 (1 col)
WA_COLS = WA_B3 + 1

F32 = mybir.dt.float32
F16 = mybir.dt.float16
AF = mybir.ActivationFunctionType
ALU = mybir.AluOpType


def build_nc() -> bass.Bass:
    nc = bacc.Bacc(
        "TRN2", target_bir_lowering=False, debug=False, num_devices=N_CORES
    )
    # Host-packed inputs (exact SBUF destination layouts):
    #   c0p[k][p][c]: c<256 -> W1eff[k*128+p, c]; c>=256 -> x_shard[c-256, k*128+p]
    #   t6e[p][c]: p<16: c<256 -> W1eff[768+p, c]; c>=256 -> x_shard[c-256, 768+p]
    #              p==16: c<256 -> b1[c]; c>=256 -> 1.0
    #   wa[p][c]: c<512 k-major W2; then k-major W3 (20); then b2 (2 cols), b3
    #   xc[ci-1][h][p][c]: c = k*512+n -> x_shard[ci*512+n, (3h+k)*128+p]
    c0p = nc.dram_tensor("c0p", [NK, 128, 256 + NCH], F16, kind="ExternalInput")
    t6e = nc.dram_tensor(
        "t6e", [KTAIL + 1, 256 + NCHUNKS * NCH], F16, kind="ExternalInput"
    )
    wa = nc.dram_tensor("wa", [128, WA_COLS], F16, kind="ExternalInput")
    xc = nc.dram_tensor(
        "xc", [NCHUNKS - 1, 2, 128, 3 * NCH], F16, kind="ExternalInput"
    )
    outT = nc.dram_tensor("outT", [NOUT, B_LOC], F16, kind="ExternalOutput")

    with tile.TileContext(nc) as tc:
        with (
            tc.tile_pool(name="wgt", bufs=1) as wp,
            tc.tile_pool(name="xin", bufs=3) as xp,
            tc.tile_pool(name="act", bufs=3) as hp,
            tc.tile_pool(name="osb", bufs=2) as op,
            tc.tile_pool(name="ps1", bufs=2, space="PSUM") as pp1,
            tc.tile_pool(name="ps2", bufs=2, space="PSUM") as pp2,
        ):
            # PE warm-up: small matmuls on a zeroed scratch tile, no DMA deps.
            warm = wp.tile([128, 128], F16, name="warm")
            nc.vector.memset(warm[:], 0.0)
            psw = pp1.tile([128, NCH], F32, name="psw", tag="ps1_0")
            for _ in range(NWARM):
                nc.tensor.matmul(
                    psw[:, 0:128], warm[:], warm[:], start=True, stop=True
                )

            # ---- chunk-0 self-contained [w|x] pieces, round-robin over the
            # three issue paths (gpsimd is alive first) ----
            c0t = [
                wp.tile([128, 256 + NCH], F16, name=f"c0p{k}") for k in range(NK)
            ]
            t6t = wp.tile([KTAIL + 1, 256 + NCHUNKS * NCH], F16, name="t6t")
            wat = wp.tile([128, WA_COLS], F16, name="wat")

            nc.gpsimd.dma_start(out=c0t[0][:], in_=c0p[0])
            nc.sync.dma_start(out=c0t[1][:], in_=c0p[1])
            nc.scalar.dma_start(out=c0t[2][:], in_=c0p[2])
            nc.gpsimd.dma_start(out=c0t[3][:], in_=c0p[3])
            nc.sync.dma_start(out=c0t[4][:], in_=c0p[4])
            nc.scalar.dma_start(out=c0t[5][:], in_=c0p[5])
            nc.gpsimd.dma_start(out=wat[:], in_=wa[:, :])
            nc.scalar.dma_start(out=t6t[:], in_=t6e[:, :])

            # later-chunk x prefetches (bufs=3 -> no sequencer blocking)
            xa_t = [None] * NCHUNKS
            xb_t = [None] * NCHUNKS
            for ci in range(1, NCHUNKS):
                xa_t[ci] = xp.tile([128, 3 * NCH], F16, name=f"xa{ci}", tag="xa")
                nc.sync.dma_start(out=xa_t[ci][:], in_=xc[ci - 1, 0])
                xb_t[ci] = xp.tile([128, 3 * NCH], F16, name=f"xb{ci}", tag="xb")
                nc.scalar.dma_start(out=xb_t[ci][:], in_=xc[ci - 1, 1])

            def w1_piece(k, m):
                if k == NK:
                    return t6t[0 : KTAIL + 1, m * 128 : (m + 1) * 128]
                return c0t[k][:, m * 128 : (m + 1) * 128]

            def x_piece(ci, k):
                if k == NK:
                    return t6t[0 : KTAIL + 1, 256 + ci * NCH : 256 + (ci + 1) * NCH]
                if ci == 0:
                    return c0t[k][:, 256 : 256 + NCH]
                if k < 3:
                    return xa_t[ci][:, k * NCH : (k + 1) * NCH]
                return xb_t[ci][:, (k - 3) * NCH : (k - 2) * NCH]

            # fp32 staging of b2/b3 (tensor_scalar needs fp32 scalar APs);
            # one DVE copy, far off the critical path.
            bf = wp.tile([128, 3], F32, name="bf")
            nc.vector.tensor_copy(bf[:], wat[:, WA_B2 : WA_B3 + 1])
            b2m = [bf[:, 0:1], bf[:, 1:2]]
            b3v = bf[0:NOUT, 2:3]

            # ---- batch-chunk pipeline ----
            for ci in range(NCHUNKS):
                n0 = ci * NCH
                last = ci == NCHUNKS - 1

                # layer 1.  k-outer/m-inner so each arriving piece feeds both
                # m matmuls at once; the LAST chunk runs m-outer so ps1_0
                # completes early and its relu overlaps the m1 pass.
                ps1 = [
                    pp1.tile([128, NCH], F32, name="ps1", tag=f"ps1_{m}")
                    for m in range(2)
                ]
                if not last:
                    for k in range(NK + 1):
                        xv = x_piece(ci, k)
                        for m in range(2):
                            nc.tensor.matmul(
                                ps1[m][:],
                                w1_piece(k, m),
                                xv,
                                start=(k == 0),
                                stop=(k == NK),
                            )
                else:
                    for m in range(2):
                        for k in range(NK + 1):
                            nc.tensor.matmul(
                                ps1[m][:],
                                w1_piece(k, m),
                                x_piece(ci, k),
                                start=(k == 0),
                                stop=(k == NK),
                            )

                h1 = []
                for m in range(2):
                    h = hp.tile([128, NCH], F16, name="h1", tag=f"h1_{m}")
                    if m == 0:
                        nc.scalar.activation(h[:], ps1[m][:], AF.Relu)
                    else:
                        nc.vector.tensor_scalar(
                            h[:], ps1[m][:], 0.0, None, ALU.max
                        )
                    h1.append(h)

                # layer 2: h2T = relu(W2.T @ h1T + b2)
                h2 = []
                for m in range(2):
                    ps = pp2.tile([128, NCH], F32, name="ps2", tag=f"ps2_{m}")
                    for k in range(2):
                        nc.tensor.matmul(
                            ps[:],
                            wat[:, k * H + m * 128 : k * H + (m + 1) * 128],
                            h1[k][:],
                            start=(k == 0),
                            stop=(k == 1),
                        )
                    h = hp.tile([128, NCH], F16, name="h2", tag=f"h2_{m}")
                    if m == 0:
                        nc.scalar.activation(h[:], ps[:], AF.Relu, bias=b2m[m])
                    else:
                        nc.vector.tensor_scalar(
                            h[:], ps[:], b2m[m], 0.0, ALU.add, ALU.max
                        )
                    h2.append(h)

                # layer 3: oT = W3.T @ h2T + b3 (shares ps2 bank slots).
                # Last chunk: two 256-col halves so bias+store pipeline.
                if not last:
                    ps3f = pp2.tile([128, NCH], F32, name="ps3", tag="ps2_1")
                    ps = ps3f[0:NOUT, :]
                    for k in range(2):
                        nc.tensor.matmul(
                            ps,
                            wat[:, WA_W3 + k * NOUT : WA_W3 + (k + 1) * NOUT],
                            h2[k][:],
                            start=(k == 0),
                            stop=(k == 1),
                        )
                    ob = op.tile([NOUT, NCH], F16, name="ob", tag="ob")
                    nc.vector.tensor_scalar(ob[:], ps, b3v, None, ALU.add)
                    nc.gpsimd.dma_start(out=outT[:, n0 : n0 + NCH], in_=ob[:])
                else:
                    for hh in range(2):
                        nlo = hh * (NCH // 2)
                        nhi = nlo + NCH // 2
                        ps3f = pp2.tile(
                            [128, NCH // 2], F32, name="ps3", tag=f"ps2_{hh}"
                        )
                        ps = ps3f[0:NOUT, :]
                        for k in range(2):
                            nc.tensor.matmul(
                                ps,
                                wat[:, WA_W3 + k * NOUT : WA_W3 + (k + 1) * NOUT],
                                h2[k][:, nlo:nhi],
                                start=(k == 0),
                                stop=(k == 1),
                            )
                        ob = op.tile([NOUT, NCH // 2], F16, name="ob", tag="ob")
                        nc.vector.tensor_scalar(ob[:], ps, b3v, None, ALU.add)
                        if hh == 0:
                            nc.gpsimd.dma_start(
                                out=outT[:, n0 + nlo : n0 + nhi], in_=ob[:]
                            )
                        else:
                            nc.sync.dma_start(
                                out=outT[:, n0 + nlo : n0 + nhi], in_=ob[:]
                            )

    nc.compile()
    return nc


def _fold_conv_into_w1(conv_w: np.ndarray, W1: np.ndarray) -> np.ndarray:
    """W1eff[784, 256] such that x @ W1eff == conv_flat(x, conv_w) @ W1."""
    W1v = W1.astype(np.float64).reshape(26, 26, W1.shape[1])
    cw = conv_w.astype(np.float64)
    acc = np.zeros((28, 28, W1.shape[1]), np.float64)
    for di in range(3):
        for dj in range(3):
            acc[di : di + 26, dj : dj + 26, :] += cw[di, dj] * W1v
    return acc.reshape(KIN, W1.shape[1]).astype(np.float32)


def _pack_kmajor(w: np.ndarray, kpad: int) -> np.ndarray:
    """[K, C] -> [128, (K/128)*C] with row-block k at column block k."""
    k, c = w.shape
    wp = np.zeros((kpad, c), w.dtype)
    wp[:k] = w
    return np.ascontiguousarray(
        wp.reshape(kpad // 128, 128, c).transpose(1, 0, 2).reshape(128, -1)
    )


def _run(inputs: dict, trace: bool = False, tmpdir: str | None = None):
    x = np.asarray(inputs["x"], dtype=np.float32)
    w1e = _fold_conv_into_w1(
        np.asarray(inputs["conv_w"]), np.asarray(inputs["W1"])
    ).astype(np.float16)
    w2P = _pack_kmajor(np.asarray(inputs["W2"], np.float16), H)
    w3P = _pack_kmajor(np.asarray(inputs["W3"], np.float16), H)
    wa = np.zeros((128, WA_COLS), np.float16)
    wa[:, : 2 * H] = w2P
    wa[:, WA_W3 : WA_W3 + 2 * NOUT] = w3P
    wa[:, WA_B2 : WA_B2 + 2] = (
        np.asarray(inputs["b2"], np.float16).reshape(2, 128).T
    )
    wa[:NOUT, WA_B3] = np.asarray(inputs["b3"], np.float16)
    b1 = np.asarray(inputs["b1"], np.float16)

    nc = build_nc()
    in_maps = []
    for c in range(N_CORES):
        xs = x[c * B_LOC : (c + 1) * B_LOC].astype(np.float16)  # [2048, 784]
        # xsT[k*128+p, n] = xs[n, k*128+p]
        xsT = np.ascontiguousarray(xs.T)  # [784, 2048]
        c0pc = np.empty((NK, 128, 256 + NCH), np.float16)
        for k in range(NK):
            c0pc[k, :, :256] = w1e[k * 128 : (k + 1) * 128]
            c0pc[k, :, 256:] = xsT[k * 128 : (k + 1) * 128, :NCH]
        t6c = np.empty((KTAIL + 1, 256 + NCHUNKS * NCH), np.float16)
        t6c[:KTAIL, :256] = w1e[NK * 128 :]
        t6c[KTAIL, :256] = b1
        t6c[:KTAIL, 256:] = xsT[NK * 128 :, :]
        t6c[KTAIL, 256:] = 1.0
        xcc = np.empty((NCHUNKS - 1, 2, 128, 3 * NCH), np.float16)
        for ci in range(1, NCHUNKS):
            for h in range(2):
                blk = xsT[
                    3 * h * 128 : 3 * (h + 1) * 128, ci * NCH : (ci + 1) * NCH
                ]  # [384, 512]
                xcc[ci - 1, h] = (
                    blk.reshape(3, 128, NCH).transpose(1, 0, 2).reshape(128, -1)
                )
        in_maps.append({"c0p": c0pc, "t6e": t6c, "wa": wa, "xc": xcc})

    try:
        res = run_bass_kernel_spmd(
            nc, in_maps, list(range(N_CORES)), trace=trace, tmpdir=tmpdir
        )
    except Exception:
        # A prior session can leave a NeuronCore wedged
        # (NRT_EXEC_UNIT_UNRECOVERABLE); a retry with core reset recovers.
        import os

        os.environ.setdefault("NEURON_RT_RESET_CORES", "1")
        res = run_bass_kernel_spmd(
            nc, in_maps, list(range(N_CORES)), trace=trace, tmpdir=tmpdir
        )
    out = np.concatenate(
        [r["outT"].astype(np.float32).T for r in res.results], axis=0
    )
    return np.ascontiguousarray(out), res


def kernel(**inputs) -> np.ndarray:
    out, _ = _run(inputs, trace=False)
    return out


# revision 8
# speedup vs baseline: 1.0669x; 1.0669x over previous
"""Trainium2 Bass kernel for nn_DigitConvolutionalModel (3x3 conv + 3-layer MLP).

Math: out = relu(relu(conv3x3(x) @ W1 + b1) @ W2 + b2) @ W3 + b3.

The 3x3 valid conv is linear, so on host we fold it into the first FC:
  h1 = relu(x @ W1eff + b1)  with  W1eff = A @ W1 : [784, 256].
K = 784 is used EXACTLY (6 full 128-row k-tiles + one 16-row tail tile).
b1 rides the tail matmul as a 17th row (ones row in x-tail, b1 row in
W-tail), so L1 needs no separate bias add.  b2/b3 ride as fp16 columns of
the wa tensor.

Sharding: pure data parallel over the batch across 8 cores (2048 rows each).
Feature-major 3-layer MLP (activations transposed; zero on-device transposes):
  h1T = relu(W1eff.T @ xT [+b1 via ones-row])   [256, 2048]
  h2T = relu(W2.T   @ h1T + b2)                 [256, 2048]
  oT  =      W3.T   @ h2T + b3                  [10, 2048]
Matmuls in fp16 (full-rate PE) with fp32 PSUM accumulation.

DMA/fill discipline (from trace analysis of the previous kernel):
 - Each dma_start costs its issuing sequencer ~0.65us of serial descriptor
   generation, and a queue's first bytes land ~0.8us after descgen ends.
   The HWDGE engines (sync/scalar) only come alive ~1.4us into the span,
   but GpSimd is alive at ~0 -- so the first chunk-0 pieces ride SWDGE.
 - Chunk-0 is split into SELF-CONTAINED per-k pieces [w1_k | x0_k] so each
   piece's semaphore releases both matmuls of that k immediately.  Pieces
   are spread round-robin over gpsimd/sync/scalar so no queue builds a
   backlog that delays another queue's first transfer.
 - The PE HAM clock gate holds the PE at 1.2GHz until ~3.4us of sustained
   busy-ness; a short warmup burst bridges engine-alive -> first data, and
   early real matmuls (still cold) keep accumulating busy time.
 - Later chunks prefetch in 3-k-tile halves on the two HWDGE rings
   (bufs=3 so no dma_start ever blocks its sequencer on buffer reuse).
 - Output is stored as fp16 (cast up on host; adds ~1e-4 rel err) and the
   last chunk's L3/bias/store is split into two 256-col halves so the
   exposed post-matmul tail is short.
"""

import numpy as np

import concourse.bacc as bacc
import concourse.bass as bass
import concourse.mybir as mybir
import concourse.tile as tile
from concourse.bass_utils import run_bass_kernel_spmd

N_CORES = 8
B = 16384
B_LOC = B // N_CORES  # 2048 batch rows per core
NCH = 512  # batch chunk per matmul (fp32 PSUM bank = 512 floats)
NCHUNKS = B_LOC // NCH
KIN = 784  # folded input features (28*28)
NK = 6  # full 128-row k-tiles; tile 6 is the 16-row tail (+1 ones row)
KTAIL = KIN - NK * 128  # 16
H = 256
NOUT = 10
NWARM = 12  # small PE warm-up matmuls bridging engine-alive -> first data

WA_W3 = 2 * H  # col offset of w3 block in wa
WA_B2 = WA_W3 + 2 * NOUT  # col offset of b2 (2 cols)
WA_B3 = WA_B2 + 2  # col offset of b3 (1 col)
WA_COLS = WA_B3 + 1

F32 = mybir.dt.float32
F16 = mybir.dt.float16
AF = mybir.ActivationFunctionType
ALU = mybir.AluOpType


def build_nc() -> bass.Bass:
    nc = bacc.Bacc(
        "TRN2", target_bir_lowering=False, debug=False, num_devices=N_CORES
    )
    # Host-packed inputs (exact SBUF destination layouts):
    #   c0p[p][k*768+c]: c<256 -> W1eff[k*128+p, c]; c>=256 -> x_shard[c-256, k*128+p]
    #   t6e[p][c]: p<16: c<256 -> W1eff[768+p, c]; c>=256 -> x_shard[c-256, 768+p]
    #              p==16: c<256 -> b1[c]; c>=256 -> 1.0
    #   wa[p][c]: c<512 k-major W2; then k-major W3 (20); then b2 (2 cols), b3
    #   xc[ci-1][h][p][c]: c = k*512+n -> x_shard[ci*512+n, (3h+k)*128+p]
    KP = 256 + NCH  # 768 cols per [w_k | x0_k] piece
    c0p = nc.dram_tensor("c0p", [128, NK * KP], F16, kind="ExternalInput")
    t6e = nc.dram_tensor(
        "t6e", [KTAIL + 1, 256 + NCHUNKS * NCH], F16, kind="ExternalInput"
    )
    wa = nc.dram_tensor("wa", [128, WA_COLS], F16, kind="ExternalInput")
    xc = nc.dram_tensor(
        "xc", [NCHUNKS - 1, 2, 128, 3 * NCH], F16, kind="ExternalInput"
    )
    outT = nc.dram_tensor("outT", [NOUT, B_LOC], F16, kind="ExternalOutput")

    with tile.TileContext(nc) as tc:
        with (
            tc.tile_pool(name="wgt", bufs=1) as wp,
            tc.tile_pool(name="xin", bufs=3) as xp,
            tc.tile_pool(name="act", bufs=3) as hp,
            tc.tile_pool(name="osb", bufs=2) as op,
            tc.tile_pool(name="ps1", bufs=2, space="PSUM") as pp1,
            tc.tile_pool(name="ps2", bufs=2, space="PSUM") as pp2,
        ):
            # PE warm-up: small matmuls on a zeroed scratch tile, no DMA deps.
            warm = wp.tile([128, 128], F16, name="warm")
            nc.vector.memset(warm[:], 0.0)
            psw = pp1.tile([128, NCH], F32, name="psw", tag="ps1_0")
            for _ in range(NWARM):
                nc.tensor.matmul(
                    psw[:, 0:128], warm[:], warm[:], start=True, stop=True
                )

            # ---- chunk-0 self-contained [w|x] pieces, alternating across the
            # two HWDGE rings in consumption order: each piece's semaphore
            # releases both matmuls of its k-tile(s) immediately ----
            tk0 = wp.tile([128, KP], F16, name="tk0")
            tk1 = wp.tile([128, KP], F16, name="tk1")
            tk23 = wp.tile([128, 2 * KP], F16, name="tk23")
            tk45 = wp.tile([128, 2 * KP], F16, name="tk45")
            t6t = wp.tile([KTAIL + 1, 256 + NCHUNKS * NCH], F16, name="t6t")
            wat = wp.tile([128, WA_COLS], F16, name="wat")

            nc.sync.dma_start(out=tk0[:], in_=c0p[:, 0:KP])
            nc.scalar.dma_start(out=tk1[:], in_=c0p[:, KP : 2 * KP])
            nc.sync.dma_start(out=tk23[:], in_=c0p[:, 2 * KP : 4 * KP])
            nc.scalar.dma_start(out=tk45[:], in_=c0p[:, 4 * KP : 6 * KP])
            nc.sync.dma_start(out=t6t[:], in_=t6e[:, :])
            nc.scalar.dma_start(out=wat[:], in_=wa[:, :])

            # later-chunk x prefetches (bufs=3 -> no sequencer blocking)
            xa_t = [None] * NCHUNKS
            xb_t = [None] * NCHUNKS
            for ci in range(1, NCHUNKS):
                xa_t[ci] = xp.tile([128, 3 * NCH], F16, name=f"xa{ci}", tag="xa")
                nc.sync.dma_start(out=xa_t[ci][:], in_=xc[ci - 1, 0])
                xb_t[ci] = xp.tile([128, 3 * NCH], F16, name=f"xb{ci}", tag="xb")
                nc.scalar.dma_start(out=xb_t[ci][:], in_=xc[ci - 1, 1])

            def c0_tile(k):
                if k == 0:
                    return tk0, 0
                if k == 1:
                    return tk1, 0
                if k < 4:
                    return tk23, (k - 2) * KP
                return tk45, (k - 4) * KP

            def w1_piece(k, m):
                if k == NK:
                    return t6t[0 : KTAIL + 1, m * 128 : (m + 1) * 128]
                t, off = c0_tile(k)
                return t[:, off + m * 128 : off + (m + 1) * 128]

            def x_piece(ci, k):
                if k == NK:
                    return t6t[0 : KTAIL + 1, 256 + ci * NCH : 256 + (ci + 1) * NCH]
                if ci == 0:
                    t, off = c0_tile(k)
                    return t[:, off + 256 : off + 256 + NCH]
                if k < 3:
                    return xa_t[ci][:, k * NCH : (k + 1) * NCH]
                return xb_t[ci][:, (k - 3) * NCH : (k - 2) * NCH]

            # fp32 staging of b2/b3 (tensor_scalar needs fp32 scalar APs);
            # one DVE copy, far off the critical path.
            bf = wp.tile([128, 3], F32, name="bf")
            nc.vector.tensor_copy(bf[:], wat[:, WA_B2 : WA_B3 + 1])
            b2m = [bf[:, 0:1], bf[:, 1:2]]
            b3v = bf[0:NOUT, 2:3]

            # ---- batch-chunk pipeline ----
            for ci in range(NCHUNKS):
                n0 = ci * NCH
                last = ci == NCHUNKS - 1

                # layer 1.  k-outer/m-inner so each arriving piece feeds both
                # m matmuls at once; the LAST chunk runs m-outer so ps1_0
                # completes early and its relu overlaps the m1 pass.
                ps1 = [
                    pp1.tile([128, NCH], F32, name="ps1", tag=f"ps1_{m}")
                    for m in range(2)
                ]
                if not last:
                    for k in range(NK + 1):
                        xv = x_piece(ci, k)
                        for m in range(2):
                            nc.tensor.matmul(
                                ps1[m][:],
                                w1_piece(k, m),
                                xv,
                                start=(k == 0),
                                stop=(k == NK),
                            )
                else:
                    for m in range(2):
                        for k in range(NK + 1):
                            nc.tensor.matmul(
                                ps1[m][:],
                                w1_piece(k, m),
                                x_piece(ci, k),
                                start=(k == 0),
                                stop=(k == NK),
                            )

                h1 = []
                for m in range(2):
                    h = hp.tile([128, NCH], F16, name="h1", tag=f"h1_{m}")
                    if m == 0:
                        nc.scalar.activation(h[:], ps1[m][:], AF.Relu)
                    else:
                        nc.vector.tensor_scalar(
                            h[:], ps1[m][:], 0.0, None, ALU.max
                        )
                    h1.append(h)

                # layer 2: h2T = relu(W2.T @ h1T + b2)
                h2 = []
                for m in range(2):
                    ps = pp2.tile([128, NCH], F32, name="ps2", tag=f"ps2_{m}")
                    for k in range(2):
                        nc.tensor.matmul(
                            ps[:],
                            wat[:, k * H + m * 128 : k * H + (m + 1) * 128],
                            h1[k][:],
                            start=(k == 0),
                            stop=(k == 1),
                        )
                    h = hp.tile([128, NCH], F16, name="h2", tag=f"h2_{m}")
                    if m == 0:
                        nc.scalar.activation(h[:], ps[:], AF.Relu, bias=b2m[m])
                    else:
                        nc.vector.tensor_scalar(
                            h[:], ps[:], b2m[m], 0.0, ALU.add, ALU.max
                        )
                    h2.append(h)

                # layer 3: oT = W3.T @ h2T + b3 (shares ps2 bank slots).
                # Last chunk: two 256-col halves so bias+store pipeline.
                if not last:
                    ps3f = pp2.tile([128, NCH], F32, name="ps3", tag="ps2_1")
                    ps = ps3f[0:NOUT, :]
                    for k in range(2):
                        nc.tensor.matmul(
                            ps,
                            wat[:, WA_W3 + k * NOUT : WA_W3 + (k + 1) * NOUT],
                            h2[k][:],
                            start=(k == 0),
                            stop=(k == 1),
                        )
                    ob = op.tile([NOUT, NCH], F16, name="ob", tag="ob")
                    nc.vector.tensor_scalar(ob[:], ps, b3v, None, ALU.add)
                    nc.gpsimd.dma_start(out=outT[:, n0 : n0 + NCH], in_=ob[:])
                else:
                    for hh in range(2):
                        nlo = hh * (NCH // 2)
                        nhi = nlo + NCH // 2
                        ps3f = pp2.tile(
                            [128, NCH // 2], F32, name="ps3", tag=f"ps2_{hh}"
                        )
                        ps = ps3f[0:NOUT, :]
                        for k in range(2):
                            nc.tensor.matmul(
                                ps,
                                wat[:, WA_W3 + k * NOUT : WA_W3 + (k + 1) * NOUT],
                                h2[k][:, nlo:nhi],
                                start=(k == 0),
                                stop=(k == 1),
                            )
                        ob = op.tile([NOUT, NCH // 2], F16, name="ob", tag="ob")
                        nc.vector.tensor_scalar(ob[:], ps, b3v, None, ALU.add)
                        if hh == 0:
                            nc.gpsimd.dma_start(
                                out=outT[:, n0 + nlo : n0 + nhi], in_=ob[:]
                            )
                        else:
                            nc.sync.dma_start(
                                out=outT[:, n0 + nlo : n0 + nhi], in_=ob[:]
                            )

    nc.compile()
    return nc


def _fold_conv_into_w1(conv_w: np.ndarray, W1: np.ndarray) -> np.ndarray:
    """W1eff[784, 256] such that x @ W1eff == conv_flat(x, conv_w) @ W1."""
    W1v = W1.astype(np.float64).reshape(26, 26, W1.shape[1])
    cw = conv_w.astype(np.float64)
    acc = np.zeros((28, 28, W1.shape[1]), np.float64)
    for di in range(3):
        for dj in range(3):
            acc[di : di + 26, dj : dj + 26, :] += cw[di, dj] * W1v
    return acc.reshape(KIN, W1.shape[1]).astype(np.float32)


def _pack_kmajor(w: np.ndarray, kpad: int) -> np.ndarray:
    """[K, C] -> [128, (K/128)*C] with row-block k at column block k."""
    k, c = w.shape
    wp = np.zeros((kpad, c), w.dtype)
    wp[:k] = w
    return np.ascontiguousarray(
        wp.reshape(kpad // 128, 128, c).transpose(1, 0, 2).reshape(128, -1)
    )


def _run(inputs: dict, trace: bool = False, tmpdir: str | None = None):
    x = np.asarray(inputs["x"], dtype=np.float32)
    w1e = _fold_conv_into_w1(
        np.asarray(inputs["conv_w"]), np.asarray(inputs["W1"])
    ).astype(np.float16)
    w2P = _pack_kmajor(np.asarray(inputs["W2"], np.float16), H)
    w3P = _pack_kmajor(np.asarray(inputs["W3"], np.float16), H)
    wa = np.zeros((128, WA_COLS), np.float16)
    wa[:, : 2 * H] = w2P
    wa[:, WA_W3 : WA_W3 + 2 * NOUT] = w3P
    wa[:, WA_B2 : WA_B2 + 2] = (
        np.asarray(inputs["b2"], np.float16).reshape(2, 128).T
    )
    wa[:NOUT, WA_B3] = np.asarray(inputs["b3"], np.float16)
    b1 = np.asarray(inputs["b1"], np.float16)

    nc = build_nc()
    in_maps = []
    for c in range(N_CORES):
        xs = x[c * B_LOC : (c + 1) * B_LOC].astype(np.float16)  # [2048, 784]
        # xsT[k*128+p, n] = xs[n, k*128+p]
        xsT = np.ascontiguousarray(xs.T)  # [784, 2048]
        KP = 256 + NCH
        c0pc = np.empty((128, NK * KP), np.float16)
        for k in range(NK):
            c0pc[:, k * KP : k * KP + 256] = w1e[k * 128 : (k + 1) * 128]
            c0pc[:, k * KP + 256 : (k + 1) * KP] = xsT[
                k * 128 : (k + 1) * 128, :NCH
            ]
        t6c = np.empty((KTAIL + 1, 256 + NCHUNKS * NCH), np.float16)
        t6c[:KTAIL, :256] = w1e[NK * 128 :]
        t6c[KTAIL, :256] = b1
        t6c[:KTAIL, 256:] = xsT[NK * 128 :, :]
        t6c[KTAIL, 256:] = 1.0
        xcc = np.empty((NCHUNKS - 1, 2, 128, 3 * NCH), np.float16)
        for ci in range(1, NCHUNKS):
            for h in range(2):
                blk = xsT[
                    3 * h * 128 : 3 * (h + 1) * 128, ci * NCH : (ci + 1) * NCH
                ]  # [384, 512]
                xcc[ci - 1, h] = (
                    blk.reshape(3, 128, NCH).transpose(1, 0, 2).reshape(128, -1)
                )
        in_maps.append({"c0p": c0pc, "t6e": t6c, "wa": wa, "xc": xcc})

    try:
        res = run_bass_kernel_spmd(
            nc, in_maps, list(range(N_CORES)), trace=trace, tmpdir=tmpdir
        )
    except Exception:
        # A prior session can leave a NeuronCore wedged
        # (NRT_EXEC_UNIT_UNRECOVERABLE); a retry with core reset recovers.
        import os

        os.environ.setdefault("NEURON_RT_RESET_CORES", "1")
        res = run_bass_kernel_spmd(
            nc, in_maps, list(range(N_CORES)), trace=trace, tmpdir=tmpdir
        )
    out = np.concatenate(
        [r["outT"].astype(np.float32).T for r in res.results], axis=0
    )
    return np.ascontiguousarray(out), res


def kernel(**inputs) -> np.ndarray:
    out, _ = _run(inputs, trace=False)
    return out
